# revision 87
# baseline (speedup 1.0000x reference)
"""Trainium2 Bass kernel for vq_codebook problem (nn_BDFR_80925773791448).

Pipeline per image:
  h    = silu(bn1(conv1x1(x)))          256->64 ch
  proj = silu(bn2(dwconv3x3(h)))        64 ch depthwise
  md   = min_k ||proj - proto_k|| / tau
  dev  = per-image minmax-normalized md
  attn = sigmoid(fuse(dsconv3(dev), dsconv5(dev)))
  out  = x * (1 + gamma * attn)

Sharding: 8 cores = 4 images x 2 row-halves (64 rows each + halo recompute).
Cross-core communication: one 2-float AllGather per pair for the per-image
min/max of md (cheaper than AllReduce in the cost model), combined locally.

Design notes (cost-model driven):
- x load (9.2MB) and out store (8.4MB) at ~360GB/s are the serial DMA floor;
  everything else pipelines under them.
- dwconv packs 2 output rows into 128 psum partitions (64ch x 2row-parity)
  -> 6 matmuls x 256 free per 4 rows instead of 6 x 512.
- distance: per-row matmul of stacked [proj; proj^2] against [-2 protos; 1],
  k on the free dim, then min-reduce.
- min/max runs on squared distances (monotone); sqrt via DVE pow 0.5 (no Act
  table switches); sigmoid via silu(x)*recip(x) (stays in the silu table).
- attention head contracts over image COLUMNS with banded matrices in the
  native [col-part, row-free] layout of md; the conv of the unnormalized md
  runs during the collective and is affinely fixed up afterwards
  (conv(dev) = R*conv(md*mask) - smin*R*conv(mask)).
- attn broadcast: fold attn to one partition, then 1-partition-contraction
  matmul with a ones row -> [128, 4x128] per output chunk.
"""

import numpy as np
import ml_dtypes

import concourse.bass as bass
import concourse.bacc as bacc
import concourse.tile as tile
import concourse.mybir as mybir
from concourse.bass_utils import run_bass_kernel_spmd

F32 = mybir.dt.float32
F32R = mybir.dt.float32r
BF16 = mybir.dt.bfloat16
AF = mybir.ActivationFunctionType
OP = mybir.AluOpType

BN_EPS = 1e-5
TAU = 1.0
EPS_R = 1e-6 * (TAU + 1e-6)

B, C, H, W = 4, 256, 128, 128
P, K = 64, 8
NCORES = 8
SLAB = 70          # x/h rows per core (64 owned + 3 halo above + 3 below)
MROWS = 68         # proj/md rows (md row m <-> image row 64*half - 2 + m)
OWN0 = 2           # md index of first owned row
NOWN = 64

# f32 blob column layout
_BL_WKA = 0
_BL_WKB = 64
_BL_DRHS = 128
_BL_PK2 = 136
_BL_CB3 = 144
_BL_CB5 = 528
_BL_BM1 = 1168
_BL_BM2 = 1232
_BL_ZM = 1296
_BL_T12 = 1298
_BL_SC = 1299
_BL_BS = 1307
_BL_FW = 1315
_BL_ID = 1323
_BL_TB = 1451   # 3 cols on rows 0:64: t_bs1, t_bl1, t2
_BL_FM = 1456   # fold matrix [128, 64]: FM[p, j] = [p==j] + [p==j+64]
_BL_ONE = 1520  # ones row: row 0 of these 128 cols = 1.0
_BL_SG = 1649   # sign column: row0 = 1, row1 = -1
NBLOB = 1650

# bf16 blob: 6 dwconv lhsT mats [128, 128] then a ones row
_BB_DWL = 0
_BB_ONES = 768
NBBF = 896

_CACHE = {}
_DEBUG = False


def _bn_fold(p):
    g, b2, m, v = p[0], p[1], p[2], p[3]
    s = g / np.sqrt(v + BN_EPS)
    t = b2 - m * s
    return s.astype(np.float64), t.astype(np.float64)


def _colband(w2d, scale):
    """[128, kh, 128] banded matrices over image columns: out = band^T-style
    contraction B[cin, dyi, cout] = w2d[dyi, cin-cout+kw//2] * scale."""
    kh, kw = w2d.shape
    r = kw // 2
    m = np.zeros((W, kh, W), np.float32)
    for cout in range(W):
        for dx in range(-r, r + 1):
            cin = cout + dx
            if 0 <= cin < W:
                m[cin, :, cout] = w2d[:, dx + r] * scale
    return m


def _maskconv(w2d, scale, half):
    """[128, NOWN] map: conv of the in-image indicator, [col, row] layout."""
    kh, kw = w2d.shape
    rr, rc = kh // 2, kw // 2
    out = np.zeros((W, NOWN), np.float32)
    for j in range(NOWN):
        img_r = 64 * half + j
        for c in range(W):
            acc = 0.0
            for dy in range(-rr, rr + 1):
                if not (0 <= img_r + dy < H):
                    continue
                for dx in range(-rc, rc + 1):
                    if 0 <= c + dx < W:
                        acc += w2d[dy + rr, dx + rc]
            out[c, j] = acc * scale
    return out


def _prep_consts(inp):
    s1, t1 = _bn_fold(np.asarray(inp["fp_bn1"], np.float64))
    w1 = np.asarray(inp["fp_w1"], np.float64)[:, :, 0, 0]          # [64, 256]
    w1f = w1 * s1[:, None]
    lhs_c1 = w1f.T.astype(np.float32)                               # [256, 64]
    xstar = np.linalg.lstsq(w1f, -t1, rcond=None)[0].astype(np.float32)

    s2, t2 = _bn_fold(np.asarray(inp["fp_bn2"], np.float64))
    dw = np.asarray(inp["fp_dw"], np.float64)[:, 0]                 # [64, 3, 3]
    # 6 dwconv lhsT mats [128, 128]: idx = Tt*3 + (dx+1), T in {-1,+1}
    # entry [copy*64+ch, rpar*64+ch] = dw[ch, dy+1, dx+1]*s2, dy = T+copy-rpar
    dwl = np.zeros((128, 6, 128), np.float32)
    ich = np.arange(64)
    for Tt, T in ((0, -1), (1, 1)):
        for dxi in range(3):
            idx = Tt * 3 + dxi
            for copy in (0, 1):
                for rpar in (0, 1):
                    dy = T + copy - rpar
                    if -1 <= dy <= 1:
                        dwl[copy * 64 + ich, idx, rpar * 64 + ich] = (
                            dw[:, dy + 1, dxi] * s2)

    protos = np.asarray(inp["protos"], np.float64)                  # [8, 64]
    drhs = np.zeros((128, K), np.float32)
    drhs[:64] = -2.0 * protos.T
    drhs[64:128] = 1.0
    pk2 = (protos * protos).sum(1).astype(np.float32)               # [8]

    s_bs1, t_bs1 = _bn_fold(np.asarray(inp["bs_bn1"], np.float64))
    s_bl1, t_bl1 = _bn_fold(np.asarray(inp["bl_bn1"], np.float64))
    w3 = np.asarray(inp["bs_dw"], np.float64)[0, 0]
    w5 = np.asarray(inp["bl_dw"], np.float64)[0, 0]
    cb3 = _colband(w3, float(s_bs1[0]))                             # [128,3,128]
    cb5 = _colband(w5, float(s_bl1[0]))                             # [128,5,128]

    s_bs2, t_bs2 = _bn_fold(np.asarray(inp["bs_bn2"], np.float64))
    s_bl2, t_bl2 = _bn_fold(np.asarray(inp["bl_bn2"], np.float64))
    pw_s = np.asarray(inp["bs_pw"], np.float64)[:, 0, 0, 0]
    pw_l = np.asarray(inp["bl_pw"], np.float64)[:, 0, 0, 0]
    a_ch = np.concatenate([pw_s * s_bs2, pw_l * s_bl2])             # [16]
    b_ch = np.concatenate([t_bs2, t_bl2])
    fw = np.asarray(inp["fuse_w"], np.float64)[0, :, 0, 0]
    fb = float(np.asarray(inp["fuse_b"], np.float64)[0])
    gamma = float(np.asarray(inp["gamma"], np.float64)[0])

    # chain pairing: op k handles channels (2k, 2k+1); k<4 from g1, else g2
    scsb = np.zeros((128, 8), np.float32)
    bsb = np.zeros((128, 8), np.float32)
    fwsb = np.zeros((128, 8), np.float32)
    for k in range(8):
        scsb[0:64, k] = a_ch[2 * k]
        scsb[64:128, k] = a_ch[2 * k + 1]
        bsb[0:64, k] = b_ch[2 * k]
        bsb[64:128, k] = b_ch[2 * k + 1]
        fwsb[0:64, k] = fw[2 * k]
        fwsb[64:128, k] = fw[2 * k + 1]

    blobs = {}
    for half in (0, 1):
        bl = np.zeros((128, NBLOB), np.float32)
        bl[:, _BL_WKA:_BL_WKA + 64] = lhs_c1[0:128]
        bl[:, _BL_WKB:_BL_WKB + 64] = lhs_c1[128:256]
        bl[:, _BL_DRHS:_BL_DRHS + 8] = drhs
        bl[:, _BL_PK2:_BL_PK2 + 8] = pk2[None, :]
        bl[:, _BL_CB3:_BL_CB3 + 384] = cb3.reshape(128, 384)
        bl[:, _BL_CB5:_BL_CB5 + 640] = cb5.reshape(128, 640)
        bl[:, _BL_BM1:_BL_BM1 + 64] = _maskconv(w3, float(s_bs1[0]), half)
        bl[:, _BL_BM2:_BL_BM2 + 64] = _maskconv(w5, float(s_bl1[0]), half)
        bl[:, _BL_ZM + 0] = 0.0 if half == 0 else 1.0   # zlo
        bl[:, _BL_ZM + 1] = 1.0 if half == 0 else 0.0   # zhi
        bl[0:64, _BL_T12] = t1.astype(np.float32)
        bl[:, _BL_SC:_BL_SC + 8] = scsb
        bl[:, _BL_BS:_BL_BS + 8] = bsb
        bl[:, _BL_FW:_BL_FW + 8] = fwsb
        bl[:, _BL_ID:_BL_ID + 128] = np.eye(128, dtype=np.float32)
        bl[0:64, _BL_TB + 0] = float(t_bs1[0])
        bl[0:64, _BL_TB + 1] = float(t_bl1[0])
        bl[0:64, _BL_TB + 2] = t2.astype(np.float32)
        ej = np.eye(64, dtype=np.float32)
        bl[0:64, _BL_FM:_BL_FM + 64] = ej
        bl[64:128, _BL_FM:_BL_FM + 64] = ej
        bl[0, _BL_ONE:_BL_ONE + 128] = 1.0
        bl[0, _BL_SG] = 1.0
        bl[1, _BL_SG] = -1.0
        blobs[half] = bl

    bbf = np.zeros((128, NBBF), np.float32)
    bbf[:, _BB_DWL:_BB_DWL + 768] = dwl.reshape(128, 768)
    bbf[0, _BB_ONES:_BB_ONES + 128] = 1.0
    bbf = bbf.astype(ml_dtypes.bfloat16)

    return dict(
        xstar=xstar, blobs=blobs, bbf=bbf,
        t_bs1=float(t_bs1[0]), t_bl1=float(t_bl1[0]),
        fb=fb, gamma=gamma,
    )


def _build_kernel(consts):
    nc = bacc.Bacc("TRN2", target_bir_lowering=False, num_devices=NCORES)

    x_sh = nc.declare_dram_parameter("x_sh", [C, SLAB, W], F32R, isOutput=False)
    blobd = nc.declare_dram_parameter("blobd", [128, NBLOB], F32R, isOutput=False)
    bbfd = nc.declare_dram_parameter("bbfd", [128, NBBF], BF16, isOutput=False)
    out_sh = nc.declare_dram_parameter("out_sh", [C, NOWN, W], F32, isOutput=True)
    if _DEBUG:
        dbg_md = nc.declare_dram_parameter("dbg_md", [128, MROWS], F32, isOutput=True)
        dbg_a1 = nc.declare_dram_parameter("dbg_a1", [128, NOWN], F32, isOutput=True)
        dbg_g1 = nc.declare_dram_parameter("dbg_g1", [128, W], F32, isOutput=True)
        dbg_zz = nc.declare_dram_parameter("dbg_zz", [NOWN, W], F32, isOutput=True)
        dbg_at = nc.declare_dram_parameter("dbg_at", [NOWN, W], F32, isOutput=True)
        dbg_af = nc.declare_dram_parameter("dbg_af", [1, NOWN * W], F32, isOutput=True)

    cc_in = nc.dram_tensor("cc_in", [2], F32)
    cc_out = nc.dram_tensor("cc_out", [4], F32)

    T_BS1 = consts["t_bs1"]
    T_BL1 = consts["t_bl1"]
    FB = consts["fb"]
    GAMMA = consts["gamma"]

    # x DMA chunks (rows, queue): queues run DMAs in parallel; finer at the
    # end for a short md tail. Queue budget: SP light early (out phase later),
    # Pool gets late rows (its queue first does the const loads).
    XCH = [(0, 4, "act"), (4, 4, "sp"), (8, 8, "sp"), (16, 8, "pool"),
           (24, 10, "sp"), (34, 12, "pool"), (46, 12, "sp"),
           (58, 6, "pool"), (64, 6, "pool")]

    with tile.TileContext(nc) as tc:
        with (
            tc.tile_pool(name="const", bufs=1) as cpool,
            tc.tile_pool(name="xbuf", bufs=1) as xpool,
            tc.tile_pool(name="hbuf", bufs=1) as hpool,
            tc.tile_pool(name="small", bufs=1) as spool,
            tc.tile_pool(name="work", bufs=3) as wpool,
            tc.tile_pool(name="outw", bufs=4) as opool,
            tc.tile_pool(name="ps_c1", bufs=2, space="PSUM") as ps_c1,
            tc.tile_pool(name="ps_dw", bufs=2, space="PSUM") as ps_dw,
            tc.tile_pool(name="ps_sm", bufs=2, space="PSUM") as ps_sm,
        ):
            # ---- constants (2 DMAs on gpsimd queue) ----
            blob = cpool.tile([128, NBLOB], F32R)
            bbf = cpool.tile([128, NBBF], BF16)
            nc.gpsimd.dma_start(out=blob[:], in_=blobd.ap())
            nc.gpsimd.dma_start(out=bbf[:], in_=bbfd.ap())
            blf = blob[:].bitcast(F32)

            wka = blob[:, _BL_WKA:_BL_WKA + 64]
            wkb = blob[:, _BL_WKB:_BL_WKB + 64]
            drhs = blob[:, _BL_DRHS:_BL_DRHS + 8]
            t1b = blf[0:64, _BL_T12:_BL_T12 + 1]
            t2b = blf[0:64, _BL_TB + 2:_BL_TB + 3]
            id128 = blob[:, _BL_ID:_BL_ID + 128].bitcast(F32)

            def dwl(idx):
                return bbf[:, _BB_DWL + 128 * idx:_BB_DWL + 128 * (idx + 1)]

            # ---- x in (multi-queue; one DMA per row chunk, both halves) ----
            xab = xpool.tile([128, 2, SLAB, W], F32R)
            xa = xab[:, 0]
            xb = xab[:, 1]
            QENG = {"sp": nc.sync, "act": nc.scalar, "pool": nc.gpsimd}
            for r0, nr, q in XCH:
                QENG[q].dma_start(out=xab[:, :, r0:r0 + nr, :],
                                  in_=x_sh.ap()[0:256, r0:r0 + nr, :])

            # ---- h buffer (lanes 64:128 = shifted +1 row), col pads ----
            h_s = hpool.tile([128, SLAB, W + 2], BF16)
            nc.vector.memset(h_s[:, :, 0:W + 2:W + 1], 0.0)

            proj = hpool.tile([128, MROWS, W], F32R)
            md = spool.tile([128, MROWS], F32R)
            mdf = md[:].bitcast(F32)
            mloc = spool.tile([128, 2], F32)

            def conv1(bb):
                # batch of 8 slab rows (last: 6)
                r0c = 8 * bb
                nrows = 6 if bb == 8 else 8
                psc = ps_c1.tile([128, 1024], F32, tag="c1", name=f"c1_{bb}")
                for s, nr in ((0, 4), (4, nrows - 4)):
                    po = psc[0:64, 512 * (s // 4):512 * (s // 4) + 128 * nr]
                    nc.tensor.matmul(po, wka,
                                     xab[:, 0, r0c + s:r0c + s + nr, :],
                                     start=True, stop=False)
                    nc.tensor.matmul(po, wkb,
                                     xab[:, 1, r0c + s:r0c + s + nr, :],
                                     start=False, stop=True)
                nc.scalar.activation(
                    out=h_s[0:64, r0c:r0c + nrows, 1:W + 1],
                    in_=psc[0:64, 0:128 * nrows], func=AF.Silu,
                    bias=t1b, scale=1.0)

            def shcopy(k):
                # h_s[64+ch, r, :] = h_s[ch, r+1, :] for rows 8k..8k+8 (last 5)
                r0c = 8 * k
                nr = 5 if k == 8 else 8
                nc.vector.tensor_copy(
                    out=h_s[64:128, r0c:r0c + nr, :],
                    in_=h_s[0:64, r0c + 1:r0c + nr + 1, :])

            pk2src = blf[:, _BL_PK2:_BL_PK2 + 8]

            def mdchunk(dc):
                # double chunk: 8 md rows (last: 4)
                m0 = 8 * dc
                nr = 4 if dc == 8 else 8
                nh = nr // 2
                pd = ps_dw.tile([128, 4, W], F32, tag="dw", name=f"dw_{dc}")
                first = True
                for Tt, T in ((0, -1), (1, 1)):
                    S = m0 + (0 if T == -1 else 2)
                    for dxi, dx in enumerate((-1, 0, 1)):
                        nc.tensor.matmul(
                            pd[:, 0:nh, :], dwl(Tt * 3 + dxi),
                            h_s[:, S:S + 2 * nh - 1:2, 1 + dx:W + 1 + dx],
                            start=first, stop=(Tt == 1 and dxi == 2))
                        first = False
                # rpar=0 -> even rows ; rpar=1 -> odd rows
                nc.scalar.activation(
                    out=proj[0:64, m0:m0 + nr:2, :], in_=pd[0:64, 0:nh, :],
                    func=AF.Silu, bias=t2b, scale=1.0)
                nc.scalar.activation(
                    out=proj[0:64, m0 + 1:m0 + nr:2, :], in_=pd[64:128, 0:nh, :],
                    func=AF.Silu, bias=t2b, scale=1.0)
                sqeng = nc.vector if dc in (1, 3, 5, 8) else nc.gpsimd
                sqeng.tensor_tensor(
                    out=proj[64:128, m0:m0 + nr, :],
                    in0=proj[0:64, m0:m0 + nr, :],
                    in1=proj[0:64, m0:m0 + nr, :], op=OP.mult)
                psd = ps_sm.tile([128, 8, K], F32, tag="dist", name=f"di_{dc}")
                for r in range(nr):
                    nc.tensor.matmul(psd[:, r, :], proj[:, m0 + r, :], drhs,
                                     start=True, stop=True)
                pk2bc = bass.AP(tensor=pk2src.tensor, offset=pk2src.offset,
                                ap=[pk2src.ap[0], [0, nr], pk2src.ap[1]])
                nc.vector.tensor_tensor(out=psd[:, 0:nr, :],
                                        in0=psd[:, 0:nr, :],
                                        in1=pk2bc, op=OP.add)
                nc.vector.tensor_reduce(
                    out=md[:, m0:m0 + nr], in_=psd[:, 0:nr, :],
                    axis=mybir.AxisListType.X, op=OP.min)
                # incremental per-column minmax over owned rows (d2 domain)
                lo = max(m0, OWN0)
                hi = min(m0 + nr, OWN0 + NOWN)
                if hi > lo:
                    cmx = spool.tile([128, 2], F32, name=f"cmx{dc}")
                    nc.vector.tensor_reduce(
                        out=cmx[:, 0:1], in_=mdf[:, lo:hi],
                        axis=mybir.AxisListType.X, op=OP.max)
                    nc.vector.tensor_reduce(
                        out=cmx[:, 1:2], in_=mdf[:, lo:hi],
                        axis=mybir.AxisListType.X, op=OP.min)
                    if dc == 0:
                        nc.vector.tensor_scalar(
                            out=mloc[:, 0:1], in0=cmx[:, 0:1], scalar1=0.0,
                            scalar2=None, op0=OP.max)
                        nc.vector.tensor_scalar(
                            out=mloc[:, 1:2], in0=cmx[:, 1:2], scalar1=0.0,
                            scalar2=None, op0=OP.max)
                    else:
                        nc.vector.tensor_tensor(
                            out=mloc[:, 0:1], in0=mloc[:, 0:1],
                            in1=cmx[:, 0:1], op=OP.max)
                        nc.vector.tensor_tensor(
                            out=mloc[:, 1:2], in0=mloc[:, 1:2],
                            in1=cmx[:, 1:2], op=OP.min)

            # ---- pipelined load/compute loop ----
            # conv1(k) ; shcopy(k-1) ; mdchunk(k-2)
            for k in range(11):
                if k <= 8:
                    conv1(k)
                if 1 <= k <= 9:
                    shcopy(k - 1)
                if k >= 2:
                    mdchunk(k - 2)

            # ---- finalize local minmax (accumulated incrementally) ----
            nc.vector.tensor_scalar(out=mloc[:], in0=mloc[:], scalar1=0.0,
                                    scalar2=None, op0=OP.max)
            nc.vector.tensor_scalar(out=mloc[:, 1:2], in0=mloc[:, 1:2],
                                    scalar1=-1.0, scalar2=None, op0=OP.mult)
            mq_ps = ps_sm.tile([128, 128], F32, tag="dist", name="mqps")
            nc.tensor.transpose(mq_ps[0:2, :], mloc[:, :], id128)
            mq = spool.tile([2, 1], F32)
            nc.vector.tensor_reduce(out=mq[:], in_=mq_ps[0:2, :],
                                    axis=mybir.AxisListType.X, op=OP.max)
            # local sqrt (monotone): [max_d2, -min_d2] -> [smax, smin]
            nc.scalar.activation(out=mq[:], in_=mq[:], func=AF.Sqrt,
                                 bias=0.0, scale=blf[0:2, _BL_SG:_BL_SG + 1])
            nc.sync.dma_start(out=cc_in.ap(), in_=mq[:])
            nc.gpsimd.collective_compute(
                "AllGather", OP.bypass,
                replica_groups=[[0, 1], [2, 3], [4, 5], [6, 7]],
                ins=[cc_in.ap()], outs=[cc_out.ap()])

            # ---- collective window: sqrt(md), halo mask, head convs ----
            # clamp d2 >= 0, then per-pixel sqrt (in the collective window)
            nc.vector.tensor_scalar(out=md[:], in0=mdf[:], scalar1=0.0,
                                    scalar2=None, op0=OP.max)
            nc.scalar.activation(out=md[:], in_=mdf[:], func=AF.Sqrt,
                                 bias=0.0, scale=1.0)
            # force the silu-table reload inside the collective window
            dsil = spool.tile([64, 1], F32)
            nc.scalar.activation(out=dsil[:], in_=t1b, func=AF.Silu,
                                 bias=0.0, scale=1.0)
            zl = blf[:, _BL_ZM:_BL_ZM + 1]
            zh = blf[:, _BL_ZM + 1:_BL_ZM + 2]
            nc.vector.tensor_scalar(out=md[:, 0:2], in0=mdf[:, 0:2],
                                    scalar1=zl, scalar2=None, op0=OP.mult)
            nc.vector.tensor_scalar(out=md[:, 66:68], in0=mdf[:, 66:68],
                                    scalar1=zh, scalar2=None, op0=OP.mult)
            a1 = ps_dw.tile([128, NOWN], F32, tag="dw", name="a1")
            for dyi in range(3):
                nc.tensor.matmul(
                    a1[:, :], blob[:, _BL_CB3 + 128 * dyi:_BL_CB3 + 128 * (dyi + 1)],
                    md[:, 1 + dyi:65 + dyi],
                    start=(dyi == 0), stop=(dyi == 2))
            a2 = ps_dw.tile([128, NOWN], F32, tag="dw", name="a2")
            for dyi in range(5):
                nc.tensor.matmul(
                    a2[:, :], blob[:, _BL_CB5 + 128 * dyi:_BL_CB5 + 128 * (dyi + 1)],
                    md[:, dyi:64 + dyi],
                    start=(dyi == 0), stop=(dyi == 4))

            # ---- post-collective tail ----
            ccb = spool.tile([128, 4], F32)
            cc_bcast = bass.AP(tensor=cc_out.ap().tensor, offset=0,
                               ap=[[0, 128], [1, 4]])
            nc.gpsimd.dma_start(out=ccb[:], in_=cc_bcast)
            # ccb = [smax0, smin0, smax1, smin1]
            sM = spool.tile([128, 2], F32)
            nc.vector.tensor_tensor(out=sM[:, 0:1], in0=ccb[:, 0:1],
                                    in1=ccb[:, 2:3], op=OP.max)
            nc.vector.tensor_tensor(out=sM[:, 1:2], in0=ccb[:, 1:2],
                                    in1=ccb[:, 3:4], op=OP.min)
            rden = spool.tile([128, 1], F32)
            nc.vector.tensor_tensor(out=rden[:], in0=sM[:, 0:1],
                                    in1=sM[:, 1:2], op=OP.subtract)
            nc.vector.tensor_scalar(out=rden[:], in0=rden[:], scalar1=EPS_R,
                                    scalar2=None, op0=OP.add)
            rr = spool.tile([128, 1], F32)
            nc.vector.reciprocal(out=rr[:], in_=rden[:])
            smr = spool.tile([128, 1], F32)
            nc.vector.tensor_tensor(out=smr[:], in0=sM[:, 1:2], in1=rr[:],
                                    op=OP.mult)

            g1d = spool.tile([128, W], F32)
            g2d = spool.tile([128, W], F32)
            for (aps, bmoff, tbi, gd) in ((a1, _BL_BM1, 0, g1d),
                                          (a2, _BL_BM2, 1, g2d)):
                tb = blf[0:64, _BL_TB + tbi:_BL_TB + tbi + 1]
                ff = wpool.tile([128, NOWN], F32, tag="fx")
                nc.vector.tensor_scalar(
                    out=ff[:], in0=blf[:, bmoff:bmoff + NOWN],
                    scalar1=smr[:], scalar2=None, op0=OP.mult)
                gg = wpool.tile([128, NOWN], F32, tag="fx")
                nc.vector.scalar_tensor_tensor(
                    out=gg[:], in0=aps[:, :], scalar=rr[:], in1=ff[:],
                    op0=OP.mult, op1=OP.subtract)
                gt = ps_sm.tile([NOWN, 128], F32, tag="dist",
                                name=f"gt{bmoff}")
                nc.tensor.transpose(gt[:, :], gg[:], id128)
                nc.scalar.activation(out=gd[0:64, :], in_=gt[:, :],
                                     func=AF.Silu, bias=tb, scale=1.0)
                nc.gpsimd.tensor_copy(out=gd[64:128, :], in_=gd[0:64, :])

            acc = spool.tile([128, W], F32R)
            for k in range(8):
                src = g1d if k < 4 else g2d
                uc = wpool.tile([128, W], F32, tag="uc")
                nc.scalar.activation(
                    out=uc[:], in_=src[:], func=AF.Silu,
                    bias=blf[:, _BL_BS + k:_BL_BS + k + 1],
                    scale=blf[:, _BL_SC + k:_BL_SC + k + 1])
                if k == 0:
                    nc.vector.tensor_scalar(
                        out=acc[:], in0=uc[:],
                        scalar1=blf[:, _BL_FW:_BL_FW + 1],
                        scalar2=None, op0=OP.mult)
                else:
                    nc.vector.scalar_tensor_tensor(
                        out=acc[:], in0=uc[:],
                        scalar=blf[:, _BL_FW + k:_BL_FW + k + 1],
                        in1=acc[:].bitcast(F32), op0=OP.mult, op1=OP.add)
            zz_ps = ps_sm.tile([NOWN, W], F32, tag="dist", name="zzps")
            nc.tensor.matmul(zz_ps[:, :], blob[:, _BL_FM:_BL_FM + 64],
                             acc[:], start=True, stop=True)
            zz = spool.tile([NOWN, W], F32)
            nc.vector.tensor_scalar(out=zz[:], in0=zz_ps[:, :], scalar1=FB,
                                    scalar2=None, op0=OP.add)
            sl = spool.tile([NOWN, W], F32)
            nc.scalar.activation(out=sl[:], in_=zz[:], func=AF.Silu,
                                 bias=0.0, scale=1.0)
            rz = spool.tile([NOWN, W], F32)
            nc.vector.reciprocal(out=rz[:], in_=zz[:])
            att = spool.tile([NOWN, W], F32)
            nc.vector.scalar_tensor_tensor(
                out=att[:], in0=sl[:], scalar=GAMMA, in1=rz[:],
                op0=OP.mult, op1=OP.mult)
            attb = spool.tile([NOWN, W], F32R)
            nc.vector.tensor_scalar(out=attb[:], in0=att[:], scalar1=1.0,
                                    scalar2=None, op0=OP.add)
            attr = spool.tile([32, 2, W], F32R)
            nc.sync.dma_start(out=attr[:], in_=attb[:])
            if _DEBUG:
                nc.sync.dma_start(out=dbg_md.ap(), in_=mdf[:])
                da1 = spool.tile([128, NOWN], F32)
                nc.vector.tensor_copy(out=da1[:], in_=a1[:, :])
                nc.sync.dma_start(out=dbg_a1.ap(), in_=da1[:])
                nc.sync.dma_start(out=dbg_g1.ap(), in_=g1d[:])
                nc.sync.dma_start(out=dbg_zz.ap(), in_=zz[:])
                nc.sync.dma_start(out=dbg_at.ap(), in_=att[:])
                daf = spool.tile([32, 2 * W], F32)
                nc.vector.tensor_copy(
                    out=daf[:],
                    in_=attr[:, :, :].rearrange("p a c -> p (a c)").bitcast(F32))
                nc.sync.dma_start(out=dbg_af.ap(), in_=daf[:])
            attrf = attr[:, :, :].rearrange("p a c -> p (a c)")

            # ---- final: out = x * attnB ----
            # even kk: fused [128,2,512] DVE mult + one 256-ch DMA
            # odd kk: oa on DVE, ob on Pool (via Act PSUM->SBUF copy)
            for kk in range(16):
                s0 = 4 * kk + 3          # x slab row of out row 4kk
                pa = ps_c1.tile([128, 512], F32, tag="c1", name=f"pa{kk}")
                for j in (0, 1):
                    src = id128[0:32, 2 * kk + j:2 * kk + j + 1].bitcast(F32R)
                    selc = bass.AP(tensor=src.tensor, offset=src.offset,
                                   ap=[src.ap[0], [0, 128]])
                    nc.tensor.matmul(pa[:, 256 * j:256 * j + 256], selc,
                                     attrf, start=True, stop=True)
                pav = pa[:, :]
                pabc = bass.AP(tensor=pav.tensor, offset=pav.offset,
                               ap=[pav.ap[0], [0, 2]] + list(pav.ap[1:]))
                oab = opool.tile([128, 2, 512], F32, tag="oab")
                nc.vector.tensor_tensor(
                    out=oab[:],
                    in0=xab[:, :, s0:s0 + 4, :].bitcast(F32),
                    in1=pabc, op=OP.mult)
                deng = (nc.sync, nc.gpsimd, nc.scalar, nc.sync)[kk % 4]
                deng.dma_start(
                    out=out_sh.ap()[0:256, 4 * kk:4 * kk + 4, :],
                    in_=oab[:].rearrange("p g (a b) -> p g a b", b=W))

    nc.compile()
    return nc


def _shard_inputs(inp, consts):
    x = np.asarray(inp["x"], np.float32)
    in_maps = []
    for j in range(NCORES):
        b, half = j // 2, j % 2
        r0 = 64 * half - 3
        slab = np.empty((C, SLAB, W), np.float32)
        slab[:] = consts["xstar"][:, None, None]
        lo, hi = max(r0, 0), min(r0 + SLAB, H)
        slab[:, lo - r0:hi - r0, :] = x[b, :, lo:hi, :]
        in_maps.append({
            "x_sh": slab,
            "blobd": consts["blobs"][half],
            "bbfd": consts["bbf"],
        })
    return in_maps


def kernel(**inputs) -> np.ndarray:
    consts = _prep_consts(inputs)
    key = "nc"
    if key not in _CACHE:
        _CACHE[key] = _build_kernel(consts)
    nc = _CACHE[key]
    in_maps = _shard_inputs(inputs, consts)
    res = run_bass_kernel_spmd(nc, in_maps, list(range(NCORES)))
    out = np.empty((B, C, H, W), np.float32)
    for j in range(NCORES):
        b, half = j // 2, j % 2
        shard = np.asarray(res.results[j]["out_sh"])
        out[b, :, 64 * half:64 * half + 64, :] = shard
    return out


# revision 89
# speedup vs baseline: 1.0040x; 1.0040x over previous
"""Trainium2 Bass kernel for vq_codebook problem (nn_BDFR_80925773791448).

Pipeline per image:
  h    = silu(bn1(conv1x1(x)))          256->64 ch
  proj = silu(bn2(dwconv3x3(h)))        64 ch depthwise
  md   = min_k ||proj - proto_k|| / tau
  dev  = per-image minmax-normalized md
  attn = sigmoid(fuse(dsconv3(dev), dsconv5(dev)))
  out  = x * (1 + gamma * attn)

Sharding: 8 cores = 4 images x 2 row-halves (64 rows each + halo recompute).
Cross-core communication: one 2-float AllGather per pair for the per-image
min/max of md (cheaper than AllReduce in the cost model), combined locally.

Design notes (cost-model driven):
- x load (9.2MB) and out store (8.4MB) at ~360GB/s are the serial DMA floor;
  everything else pipelines under them.
- dwconv packs 2 output rows into 128 psum partitions (64ch x 2row-parity)
  -> 6 matmuls x 256 free per 4 rows instead of 6 x 512.
- distance: per-row matmul of stacked [proj; proj^2] against [-2 protos; 1],
  k on the free dim, then min-reduce.
- min/max runs on squared distances (monotone) and accumulates incrementally
  in per-chunk slack; scalar sqrts happen pre-collective (Act sqrt with a
  +/-1 per-partition scale); the per-pixel sqrt, its table loads, and the
  banded head-conv matmuls all hide inside the collective window.
- attention head contracts over image COLUMNS with banded matrices in the
  native [col-part, row-free] layout of md; the conv of the unnormalized md
  runs during the collective and is affinely fixed up afterwards
  (conv(dev) = R*conv(md*mask) - smin*R*conv(mask)).
- sigmoid via silu(x)*recip(x) so the Act engine never leaves the silu table.
- attn broadcast: fold attn to [32, 2, W] (cheap partition-count-friendly
  DMA), then per-chunk sel matmuls whose lhsT is a stride-0-broadcast
  identity column -> [128, 4x128], no selector constant needed.
- measured dead ends (do not retry blindly): remote_dma compiles (needs
  tile_critical + >=2-col f32r matmuls + no dual-PSUM-operand DVE ops) but
  the execution backend rejects it at runtime; GPSIMD cannot read PSUM; Act
  Copy/Identity from PSUM corrupts data on the real ISA path; DVE pow is
  not valid ISA.
"""

import numpy as np
import ml_dtypes

import concourse.bass as bass
import concourse.bacc as bacc
import concourse.tile as tile
import concourse.mybir as mybir
from concourse.bass_utils import run_bass_kernel_spmd

F32 = mybir.dt.float32
F32R = mybir.dt.float32r
BF16 = mybir.dt.bfloat16
AF = mybir.ActivationFunctionType
OP = mybir.AluOpType

BN_EPS = 1e-5
TAU = 1.0
EPS_R = 1e-6 * (TAU + 1e-6)

B, C, H, W = 4, 256, 128, 128
P, K = 64, 8
NCORES = 8
SLAB = 70          # x/h rows per core (64 owned + 3 halo above + 3 below)
MROWS = 68         # proj/md rows (md row m <-> image row 64*half - 2 + m)
OWN0 = 2           # md index of first owned row
NOWN = 64

# f32 blob column layout
_BL_WKA = 0
_BL_WKB = 64
_BL_DRHS = 128
_BL_PK2 = 136
_BL_CB3 = 144
_BL_CB5 = 528
_BL_BM1 = 1168
_BL_BM2 = 1232
_BL_ZM = 1296
_BL_T12 = 1298
_BL_SC = 1299
_BL_BS = 1307
_BL_FW = 1315
_BL_ID = 1323
_BL_TB = 1451   # 3 cols on rows 0:64: t_bs1, t_bl1, t2
_BL_FM = 1456   # fold matrix [128, 64]: FM[p, j] = [p==j] + [p==j+64]
_BL_ONE = 1520  # ones row: row 0 of these 128 cols = 1.0
_BL_SG = 1649   # sign column: row0 = 1, row1 = -1
NBLOB = 1650

# bf16 blob: 6 dwconv lhsT mats [128, 128] then a ones row
_BB_DWL = 0
_BB_ONES = 768
NBBF = 896

_CACHE = {}
_DEBUG = False


def _bn_fold(p):
    g, b2, m, v = p[0], p[1], p[2], p[3]
    s = g / np.sqrt(v + BN_EPS)
    t = b2 - m * s
    return s.astype(np.float64), t.astype(np.float64)


def _colband(w2d, scale):
    """[128, kh, 128] banded matrices over image columns: out = band^T-style
    contraction B[cin, dyi, cout] = w2d[dyi, cin-cout+kw//2] * scale."""
    kh, kw = w2d.shape
    r = kw // 2
    m = np.zeros((W, kh, W), np.float32)
    for cout in range(W):
        for dx in range(-r, r + 1):
            cin = cout + dx
            if 0 <= cin < W:
                m[cin, :, cout] = w2d[:, dx + r] * scale
    return m


def _maskconv(w2d, scale, half):
    """[128, NOWN] map: conv of the in-image indicator, [col, row] layout."""
    kh, kw = w2d.shape
    rr, rc = kh // 2, kw // 2
    out = np.zeros((W, NOWN), np.float32)
    for j in range(NOWN):
        img_r = 64 * half + j
        for c in range(W):
            acc = 0.0
            for dy in range(-rr, rr + 1):
                if not (0 <= img_r + dy < H):
                    continue
                for dx in range(-rc, rc + 1):
                    if 0 <= c + dx < W:
                        acc += w2d[dy + rr, dx + rc]
            out[c, j] = acc * scale
    return out


def _prep_consts(inp):
    s1, t1 = _bn_fold(np.asarray(inp["fp_bn1"], np.float64))
    w1 = np.asarray(inp["fp_w1"], np.float64)[:, :, 0, 0]          # [64, 256]
    w1f = w1 * s1[:, None]
    lhs_c1 = w1f.T.astype(np.float32)                               # [256, 64]
    xstar = np.linalg.lstsq(w1f, -t1, rcond=None)[0].astype(np.float32)

    s2, t2 = _bn_fold(np.asarray(inp["fp_bn2"], np.float64))
    dw = np.asarray(inp["fp_dw"], np.float64)[:, 0]                 # [64, 3, 3]
    # 6 dwconv lhsT mats [128, 128]: idx = Tt*3 + (dx+1), T in {-1,+1}
    # entry [copy*64+ch, rpar*64+ch] = dw[ch, dy+1, dx+1]*s2, dy = T+copy-rpar
    dwl = np.zeros((128, 6, 128), np.float32)
    ich = np.arange(64)
    for Tt, T in ((0, -1), (1, 1)):
        for dxi in range(3):
            idx = Tt * 3 + dxi
            for copy in (0, 1):
                for rpar in (0, 1):
                    dy = T + copy - rpar
                    if -1 <= dy <= 1:
                        dwl[copy * 64 + ich, idx, rpar * 64 + ich] = (
                            dw[:, dy + 1, dxi] * s2)

    protos = np.asarray(inp["protos"], np.float64)                  # [8, 64]
    drhs = np.zeros((128, K), np.float32)
    drhs[:64] = -2.0 * protos.T
    drhs[64:128] = 1.0
    pk2 = (protos * protos).sum(1).astype(np.float32)               # [8]

    s_bs1, t_bs1 = _bn_fold(np.asarray(inp["bs_bn1"], np.float64))
    s_bl1, t_bl1 = _bn_fold(np.asarray(inp["bl_bn1"], np.float64))
    w3 = np.asarray(inp["bs_dw"], np.float64)[0, 0]
    w5 = np.asarray(inp["bl_dw"], np.float64)[0, 0]
    cb3 = _colband(w3, float(s_bs1[0]))                             # [128,3,128]
    cb5 = _colband(w5, float(s_bl1[0]))                             # [128,5,128]

    s_bs2, t_bs2 = _bn_fold(np.asarray(inp["bs_bn2"], np.float64))
    s_bl2, t_bl2 = _bn_fold(np.asarray(inp["bl_bn2"], np.float64))
    pw_s = np.asarray(inp["bs_pw"], np.float64)[:, 0, 0, 0]
    pw_l = np.asarray(inp["bl_pw"], np.float64)[:, 0, 0, 0]
    a_ch = np.concatenate([pw_s * s_bs2, pw_l * s_bl2])             # [16]
    b_ch = np.concatenate([t_bs2, t_bl2])
    fw = np.asarray(inp["fuse_w"], np.float64)[0, :, 0, 0]
    fb = float(np.asarray(inp["fuse_b"], np.float64)[0])
    gamma = float(np.asarray(inp["gamma"], np.float64)[0])

    # chain pairing: op k handles channels (2k, 2k+1); k<4 from g1, else g2
    scsb = np.zeros((128, 8), np.float32)
    bsb = np.zeros((128, 8), np.float32)
    fwsb = np.zeros((128, 8), np.float32)
    for k in range(8):
        scsb[0:64, k] = a_ch[2 * k]
        scsb[64:128, k] = a_ch[2 * k + 1]
        bsb[0:64, k] = b_ch[2 * k]
        bsb[64:128, k] = b_ch[2 * k + 1]
        fwsb[0:64, k] = fw[2 * k]
        fwsb[64:128, k] = fw[2 * k + 1]

    blobs = {}
    for half in (0, 1):
        bl = np.zeros((128, NBLOB), np.float32)
        bl[:, _BL_WKA:_BL_WKA + 64] = lhs_c1[0:128]
        bl[:, _BL_WKB:_BL_WKB + 64] = lhs_c1[128:256]
        bl[:, _BL_DRHS:_BL_DRHS + 8] = drhs
        bl[:, _BL_PK2:_BL_PK2 + 8] = pk2[None, :]
        bl[:, _BL_CB3:_BL_CB3 + 384] = cb3.reshape(128, 384)
        bl[:, _BL_CB5:_BL_CB5 + 640] = cb5.reshape(128, 640)
        bl[:, _BL_BM1:_BL_BM1 + 64] = _maskconv(w3, float(s_bs1[0]), half)
        bl[:, _BL_BM2:_BL_BM2 + 64] = _maskconv(w5, float(s_bl1[0]), half)
        bl[:, _BL_ZM + 0] = 0.0 if half == 0 else 1.0   # zlo
        bl[:, _BL_ZM + 1] = 1.0 if half == 0 else 0.0   # zhi
        bl[0:64, _BL_T12] = t1.astype(np.float32)
        bl[:, _BL_SC:_BL_SC + 8] = scsb
        bl[:, _BL_BS:_BL_BS + 8] = bsb
        bl[:, _BL_FW:_BL_FW + 8] = fwsb
        bl[:, _BL_ID:_BL_ID + 128] = np.eye(128, dtype=np.float32)
        bl[0:64, _BL_TB + 0] = float(t_bs1[0])
        bl[0:64, _BL_TB + 1] = float(t_bl1[0])
        bl[0:64, _BL_TB + 2] = t2.astype(np.float32)
        ej = np.eye(64, dtype=np.float32)
        bl[0:64, _BL_FM:_BL_FM + 64] = ej
        bl[64:128, _BL_FM:_BL_FM + 64] = ej
        bl[0, _BL_ONE:_BL_ONE + 128] = 1.0
        bl[0, _BL_SG] = 1.0
        bl[1, _BL_SG] = -1.0
        blobs[half] = bl

    bbf = np.zeros((128, NBBF), np.float32)
    bbf[:, _BB_DWL:_BB_DWL + 768] = dwl.reshape(128, 768)
    bbf[0, _BB_ONES:_BB_ONES + 128] = 1.0
    bbf = bbf.astype(ml_dtypes.bfloat16)

    return dict(
        xstar=xstar, blobs=blobs, bbf=bbf,
        t_bs1=float(t_bs1[0]), t_bl1=float(t_bl1[0]),
        fb=fb, gamma=gamma,
    )


def _build_kernel(consts):
    nc = bacc.Bacc("TRN2", target_bir_lowering=False, num_devices=NCORES)

    x_sh = nc.declare_dram_parameter("x_sh", [C, SLAB, W], F32R, isOutput=False)
    blobd = nc.declare_dram_parameter("blobd", [128, NBLOB], F32R, isOutput=False)
    bbfd = nc.declare_dram_parameter("bbfd", [128, NBBF], BF16, isOutput=False)
    out_sh = nc.declare_dram_parameter("out_sh", [C, NOWN, W], F32, isOutput=True)
    if _DEBUG:
        dbg_md = nc.declare_dram_parameter("dbg_md", [128, MROWS], F32, isOutput=True)
        dbg_a1 = nc.declare_dram_parameter("dbg_a1", [128, NOWN], F32, isOutput=True)
        dbg_g1 = nc.declare_dram_parameter("dbg_g1", [128, W], F32, isOutput=True)
        dbg_zz = nc.declare_dram_parameter("dbg_zz", [NOWN, W], F32, isOutput=True)
        dbg_at = nc.declare_dram_parameter("dbg_at", [NOWN, W], F32, isOutput=True)
        dbg_af = nc.declare_dram_parameter("dbg_af", [1, NOWN * W], F32, isOutput=True)

    cc_in = nc.dram_tensor("cc_in", [2], F32)
    cc_out = nc.dram_tensor("cc_out", [4], F32)

    T_BS1 = consts["t_bs1"]
    T_BL1 = consts["t_bl1"]
    FB = consts["fb"]
    GAMMA = consts["gamma"]

    # x DMA chunks (rows, queue): queues run DMAs in parallel; finer at the
    # end for a short md tail. Queue budget: SP light early (out phase later),
    # Pool gets late rows (its queue first does the const loads).
    XCH = [(0, 4, "act"), (4, 4, "sp"), (8, 8, "sp"), (16, 8, "pool"),
           (24, 10, "sp"), (34, 12, "pool"), (46, 12, "sp"),
           (58, 6, "pool"), (64, 6, "pool")]

    with tile.TileContext(nc) as tc:
        with (
            tc.tile_pool(name="const", bufs=1) as cpool,
            tc.tile_pool(name="xbuf", bufs=1) as xpool,
            tc.tile_pool(name="hbuf", bufs=1) as hpool,
            tc.tile_pool(name="small", bufs=1) as spool,
            tc.tile_pool(name="work", bufs=3) as wpool,
            tc.tile_pool(name="outw", bufs=4) as opool,
            tc.tile_pool(name="ps_c1", bufs=2, space="PSUM") as ps_c1,
            tc.tile_pool(name="ps_dw", bufs=2, space="PSUM") as ps_dw,
            tc.tile_pool(name="ps_sm", bufs=2, space="PSUM") as ps_sm,
        ):
            # ---- constants (2 DMAs on gpsimd queue) ----
            blob = cpool.tile([128, NBLOB], F32R)
            bbf = cpool.tile([128, NBBF], BF16)
            nc.gpsimd.dma_start(out=blob[:], in_=blobd.ap())
            nc.gpsimd.dma_start(out=bbf[:], in_=bbfd.ap())
            blf = blob[:].bitcast(F32)

            wka = blob[:, _BL_WKA:_BL_WKA + 64]
            wkb = blob[:, _BL_WKB:_BL_WKB + 64]
            drhs = blob[:, _BL_DRHS:_BL_DRHS + 8]
            t1b = blf[0:64, _BL_T12:_BL_T12 + 1]
            t2b = blf[0:64, _BL_TB + 2:_BL_TB + 3]
            id128 = blob[:, _BL_ID:_BL_ID + 128].bitcast(F32)

            def dwl(idx):
                return bbf[:, _BB_DWL + 128 * idx:_BB_DWL + 128 * (idx + 1)]

            # ---- x in (multi-queue; one DMA per row chunk, both halves) ----
            xab = xpool.tile([128, 2, SLAB, W], F32R)
            xa = xab[:, 0]
            xb = xab[:, 1]
            QENG = {"sp": nc.sync, "act": nc.scalar, "pool": nc.gpsimd}
            for r0, nr, q in XCH:
                QENG[q].dma_start(out=xab[:, :, r0:r0 + nr, :],
                                  in_=x_sh.ap()[0:256, r0:r0 + nr, :])

            # ---- h buffer (lanes 64:128 = shifted +1 row), col pads ----
            h_s = hpool.tile([128, SLAB, W + 2], BF16)
            nc.vector.memset(h_s[:, :, 0:W + 2:W + 1], 0.0)

            proj = hpool.tile([128, MROWS, W], F32R)
            md = spool.tile([128, MROWS], F32R)
            mdf = md[:].bitcast(F32)
            mloc = spool.tile([128, 2], F32)

            def conv1(bb):
                # batch of 8 slab rows (last: 6)
                r0c = 8 * bb
                nrows = 6 if bb == 8 else 8
                psc = ps_c1.tile([128, 1024], F32, tag="c1", name=f"c1_{bb}")
                for s, nr in ((0, 4), (4, nrows - 4)):
                    po = psc[0:64, 512 * (s // 4):512 * (s // 4) + 128 * nr]
                    nc.tensor.matmul(po, wka,
                                     xab[:, 0, r0c + s:r0c + s + nr, :],
                                     start=True, stop=False)
                    nc.tensor.matmul(po, wkb,
                                     xab[:, 1, r0c + s:r0c + s + nr, :],
                                     start=False, stop=True)
                nc.scalar.activation(
                    out=h_s[0:64, r0c:r0c + nrows, 1:W + 1],
                    in_=psc[0:64, 0:128 * nrows], func=AF.Silu,
                    bias=t1b, scale=1.0)

            def shcopy(k):
                # h_s[64+ch, r, :] = h_s[ch, r+1, :] for rows 8k..8k+8 (last 5)
                r0c = 8 * k
                nr = 5 if k == 8 else 8
                nc.vector.tensor_copy(
                    out=h_s[64:128, r0c:r0c + nr, :],
                    in_=h_s[0:64, r0c + 1:r0c + nr + 1, :])

            pk2src = blf[:, _BL_PK2:_BL_PK2 + 8]

            def mdchunk(dc):
                # double chunk: 8 md rows (last: 4)
                m0 = 8 * dc
                nr = 4 if dc == 8 else 8
                nh = nr // 2
                pd = ps_dw.tile([128, 4, W], F32, tag="dw", name=f"dw_{dc}")
                first = True
                for Tt, T in ((0, -1), (1, 1)):
                    S = m0 + (0 if T == -1 else 2)
                    for dxi, dx in enumerate((-1, 0, 1)):
                        nc.tensor.matmul(
                            pd[:, 0:nh, :], dwl(Tt * 3 + dxi),
                            h_s[:, S:S + 2 * nh - 1:2, 1 + dx:W + 1 + dx],
                            start=first, stop=(Tt == 1 and dxi == 2))
                        first = False
                # rpar=0 -> even rows ; rpar=1 -> odd rows
                nc.scalar.activation(
                    out=proj[0:64, m0:m0 + nr:2, :], in_=pd[0:64, 0:nh, :],
                    func=AF.Silu, bias=t2b, scale=1.0)
                nc.scalar.activation(
                    out=proj[0:64, m0 + 1:m0 + nr:2, :], in_=pd[64:128, 0:nh, :],
                    func=AF.Silu, bias=t2b, scale=1.0)
                sqeng = nc.vector if dc in (1, 3, 5) else nc.gpsimd
                sqeng.tensor_tensor(
                    out=proj[64:128, m0:m0 + nr, :],
                    in0=proj[0:64, m0:m0 + nr, :],
                    in1=proj[0:64, m0:m0 + nr, :], op=OP.mult)
                psd = ps_sm.tile([128, 8, K], F32, tag="dist", name=f"di_{dc}")
                for r in range(nr):
                    nc.tensor.matmul(psd[:, r, :], proj[:, m0 + r, :], drhs,
                                     start=True, stop=True)
                pk2bc = bass.AP(tensor=pk2src.tensor, offset=pk2src.offset,
                                ap=[pk2src.ap[0], [0, nr], pk2src.ap[1]])
                nc.vector.tensor_tensor(out=psd[:, 0:nr, :],
                                        in0=psd[:, 0:nr, :],
                                        in1=pk2bc, op=OP.add)
                nc.vector.tensor_reduce(
                    out=md[:, m0:m0 + nr], in_=psd[:, 0:nr, :],
                    axis=mybir.AxisListType.X, op=OP.min)
                # incremental per-column minmax over owned rows (d2 domain)
                lo = max(m0, OWN0)
                hi = min(m0 + nr, OWN0 + NOWN)
                if hi > lo:
                    cmx = spool.tile([128, 2], F32, name=f"cmx{dc}")
                    nc.vector.tensor_reduce(
                        out=cmx[:, 0:1], in_=mdf[:, lo:hi],
                        axis=mybir.AxisListType.X, op=OP.max)
                    nc.vector.tensor_reduce(
                        out=cmx[:, 1:2], in_=mdf[:, lo:hi],
                        axis=mybir.AxisListType.X, op=OP.min)
                    if dc == 0:
                        nc.vector.tensor_scalar(
                            out=mloc[:, 0:1], in0=cmx[:, 0:1], scalar1=0.0,
                            scalar2=None, op0=OP.max)
                        nc.vector.tensor_scalar(
                            out=mloc[:, 1:2], in0=cmx[:, 1:2], scalar1=0.0,
                            scalar2=None, op0=OP.max)
                    else:
                        nc.vector.tensor_tensor(
                            out=mloc[:, 0:1], in0=mloc[:, 0:1],
                            in1=cmx[:, 0:1], op=OP.max)
                        nc.vector.tensor_tensor(
                            out=mloc[:, 1:2], in0=mloc[:, 1:2],
                            in1=cmx[:, 1:2], op=OP.min)

            # ---- pipelined load/compute loop ----
            # conv1(k) ; shcopy(k-1) ; mdchunk(k-2)
            for k in range(11):
                if k <= 8:
                    conv1(k)
                if 1 <= k <= 9:
                    shcopy(k - 1)
                if k >= 2:
                    mdchunk(k - 2)

            # ---- finalize local minmax (accumulated incrementally) ----
            nc.vector.tensor_scalar(out=mloc[:], in0=mloc[:], scalar1=0.0,
                                    scalar2=None, op0=OP.max)
            nc.vector.tensor_scalar(out=mloc[:, 1:2], in0=mloc[:, 1:2],
                                    scalar1=-1.0, scalar2=None, op0=OP.mult)
            mq_ps = ps_sm.tile([128, 128], F32, tag="dist", name="mqps")
            nc.tensor.transpose(mq_ps[0:2, :], mloc[:, :], id128)
            mq = spool.tile([2, 1], F32)
            nc.vector.tensor_reduce(out=mq[:], in_=mq_ps[0:2, :],
                                    axis=mybir.AxisListType.X, op=OP.max)
            # local sqrt (monotone): [max_d2, -min_d2] -> [smax, smin]
            nc.scalar.activation(out=mq[:], in_=mq[:], func=AF.Sqrt,
                                 bias=0.0, scale=blf[0:2, _BL_SG:_BL_SG + 1])
            nc.sync.dma_start(out=cc_in.ap(), in_=mq[:])
            nc.gpsimd.collective_compute(
                "AllGather", OP.bypass,
                replica_groups=[[0, 1], [2, 3], [4, 5], [6, 7]],
                ins=[cc_in.ap()], outs=[cc_out.ap()])

            # ---- collective window: sqrt(md), halo mask, head convs ----
            # clamp d2 >= 0, then per-pixel sqrt (in the collective window)
            nc.vector.tensor_scalar(out=md[:], in0=mdf[:], scalar1=0.0,
                                    scalar2=None, op0=OP.max)
            nc.scalar.activation(out=md[:], in_=mdf[:], func=AF.Sqrt,
                                 bias=0.0, scale=1.0)
            # force the silu-table reload inside the collective window
            dsil = spool.tile([64, 1], F32)
            nc.scalar.activation(out=dsil[:], in_=t1b, func=AF.Silu,
                                 bias=0.0, scale=1.0)
            zl = blf[:, _BL_ZM:_BL_ZM + 1]
            zh = blf[:, _BL_ZM + 1:_BL_ZM + 2]
            nc.vector.tensor_scalar(out=md[:, 0:2], in0=mdf[:, 0:2],
                                    scalar1=zl, scalar2=None, op0=OP.mult)
            nc.vector.tensor_scalar(out=md[:, 66:68], in0=mdf[:, 66:68],
                                    scalar1=zh, scalar2=None, op0=OP.mult)
            a1 = ps_dw.tile([128, NOWN], F32, tag="dw", name="a1")
            for dyi in range(3):
                nc.tensor.matmul(
                    a1[:, :], blob[:, _BL_CB3 + 128 * dyi:_BL_CB3 + 128 * (dyi + 1)],
                    md[:, 1 + dyi:65 + dyi],
                    start=(dyi == 0), stop=(dyi == 2))
            a2 = ps_dw.tile([128, NOWN], F32, tag="dw", name="a2")
            for dyi in range(5):
                nc.tensor.matmul(
                    a2[:, :], blob[:, _BL_CB5 + 128 * dyi:_BL_CB5 + 128 * (dyi + 1)],
                    md[:, dyi:64 + dyi],
                    start=(dyi == 0), stop=(dyi == 4))

            # ---- post-collective tail ----
            ccb = spool.tile([128, 4], F32)
            cc_bcast = bass.AP(tensor=cc_out.ap().tensor, offset=0,
                               ap=[[0, 128], [1, 4]])
            nc.gpsimd.dma_start(out=ccb[:], in_=cc_bcast)
            # ccb = [smax0, smin0, smax1, smin1]
            sM = spool.tile([128, 2], F32)
            nc.vector.tensor_tensor(out=sM[:, 0:1], in0=ccb[:, 0:1],
                                    in1=ccb[:, 2:3], op=OP.max)
            nc.vector.tensor_tensor(out=sM[:, 1:2], in0=ccb[:, 1:2],
                                    in1=ccb[:, 3:4], op=OP.min)
            rden = spool.tile([128, 1], F32)
            nc.vector.tensor_tensor(out=rden[:], in0=sM[:, 0:1],
                                    in1=sM[:, 1:2], op=OP.subtract)
            nc.vector.tensor_scalar(out=rden[:], in0=rden[:], scalar1=EPS_R,
                                    scalar2=None, op0=OP.add)
            rr = spool.tile([128, 1], F32)
            nc.vector.reciprocal(out=rr[:], in_=rden[:])
            smr = spool.tile([128, 1], F32)
            nc.vector.tensor_tensor(out=smr[:], in0=sM[:, 1:2], in1=rr[:],
                                    op=OP.mult)

            g1d = spool.tile([128, W], F32)
            g2d = spool.tile([128, W], F32)
            for (aps, bmoff, tbi, gd) in ((a1, _BL_BM1, 0, g1d),
                                          (a2, _BL_BM2, 1, g2d)):
                tb = blf[0:64, _BL_TB + tbi:_BL_TB + tbi + 1]
                ff = wpool.tile([128, NOWN], F32, tag="fx")
                nc.vector.tensor_scalar(
                    out=ff[:], in0=blf[:, bmoff:bmoff + NOWN],
                    scalar1=smr[:], scalar2=None, op0=OP.mult)
                gg = wpool.tile([128, NOWN], F32, tag="fx")
                nc.vector.scalar_tensor_tensor(
                    out=gg[:], in0=aps[:, :], scalar=rr[:], in1=ff[:],
                    op0=OP.mult, op1=OP.subtract)
                gt = ps_sm.tile([NOWN, 128], F32, tag="dist",
                                name=f"gt{bmoff}")
                nc.tensor.transpose(gt[:, :], gg[:], id128)
                nc.scalar.activation(out=gd[0:64, :], in_=gt[:, :],
                                     func=AF.Silu, bias=tb, scale=1.0)
                nc.gpsimd.tensor_copy(out=gd[64:128, :], in_=gd[0:64, :])

            acc = spool.tile([128, W], F32R)
            for k in range(8):
                src = g1d if k < 4 else g2d
                uc = wpool.tile([128, W], F32, tag="uc")
                nc.scalar.activation(
                    out=uc[:], in_=src[:], func=AF.Silu,
                    bias=blf[:, _BL_BS + k:_BL_BS + k + 1],
                    scale=blf[:, _BL_SC + k:_BL_SC + k + 1])
                if k == 0:
                    nc.vector.tensor_scalar(
                        out=acc[:], in0=uc[:],
                        scalar1=blf[:, _BL_FW:_BL_FW + 1],
                        scalar2=None, op0=OP.mult)
                else:
                    nc.vector.scalar_tensor_tensor(
                        out=acc[:], in0=uc[:],
                        scalar=blf[:, _BL_FW + k:_BL_FW + k + 1],
                        in1=acc[:].bitcast(F32), op0=OP.mult, op1=OP.add)
            zz_ps = ps_sm.tile([NOWN, W], F32, tag="dist", name="zzps")
            nc.tensor.matmul(zz_ps[:, :], blob[:, _BL_FM:_BL_FM + 64],
                             acc[:], start=True, stop=True)
            zz = spool.tile([NOWN, W], F32)
            nc.vector.tensor_scalar(out=zz[:], in0=zz_ps[:, :], scalar1=FB,
                                    scalar2=None, op0=OP.add)
            sl = spool.tile([NOWN, W], F32)
            nc.scalar.activation(out=sl[:], in_=zz[:], func=AF.Silu,
                                 bias=0.0, scale=1.0)
            rz = spool.tile([NOWN, W], F32)
            nc.vector.reciprocal(out=rz[:], in_=zz[:])
            att = spool.tile([NOWN, W], F32)
            nc.vector.scalar_tensor_tensor(
                out=att[:], in0=sl[:], scalar=GAMMA, in1=rz[:],
                op0=OP.mult, op1=OP.mult)
            attb = spool.tile([NOWN, W], F32R)
            nc.vector.tensor_scalar(out=attb[:], in0=att[:], scalar1=1.0,
                                    scalar2=None, op0=OP.add)
            attr = spool.tile([32, 2, W], F32R)
            nc.sync.dma_start(out=attr[:], in_=attb[:])
            if _DEBUG:
                nc.sync.dma_start(out=dbg_md.ap(), in_=mdf[:])
                da1 = spool.tile([128, NOWN], F32)
                nc.vector.tensor_copy(out=da1[:], in_=a1[:, :])
                nc.sync.dma_start(out=dbg_a1.ap(), in_=da1[:])
                nc.sync.dma_start(out=dbg_g1.ap(), in_=g1d[:])
                nc.sync.dma_start(out=dbg_zz.ap(), in_=zz[:])
                nc.sync.dma_start(out=dbg_at.ap(), in_=att[:])
                daf = spool.tile([32, 2 * W], F32)
                nc.vector.tensor_copy(
                    out=daf[:],
                    in_=attr[:, :, :].rearrange("p a c -> p (a c)").bitcast(F32))
                nc.sync.dma_start(out=dbg_af.ap(), in_=daf[:])
            attrf = attr[:, :, :].rearrange("p a c -> p (a c)")

            # ---- final: out = x * attnB ----
            # even kk: fused [128,2,512] DVE mult + one 256-ch DMA
            # odd kk: oa on DVE, ob on Pool (via Act PSUM->SBUF copy)
            for kk in range(16):
                s0 = 4 * kk + 3          # x slab row of out row 4kk
                pa = ps_c1.tile([128, 512], F32, tag="c1", name=f"pa{kk}")
                for j in (0, 1):
                    src = id128[0:32, 2 * kk + j:2 * kk + j + 1].bitcast(F32R)
                    selc = bass.AP(tensor=src.tensor, offset=src.offset,
                                   ap=[src.ap[0], [0, 128]])
                    nc.tensor.matmul(pa[:, 256 * j:256 * j + 256], selc,
                                     attrf, start=True, stop=True)
                pav = pa[:, :]
                pabc = bass.AP(tensor=pav.tensor, offset=pav.offset,
                               ap=[pav.ap[0], [0, 2]] + list(pav.ap[1:]))
                oab = opool.tile([128, 2, 512], F32, tag="oab")
                nc.vector.tensor_tensor(
                    out=oab[:],
                    in0=xab[:, :, s0:s0 + 4, :].bitcast(F32),
                    in1=pabc, op=OP.mult)
                deng = (nc.sync, nc.gpsimd, nc.scalar, nc.sync)[kk % 4]
                deng.dma_start(
                    out=out_sh.ap()[0:256, 4 * kk:4 * kk + 4, :],
                    in_=oab[:].rearrange("p g (a b) -> p g a b", b=W))

    nc.compile()
    return nc


def _shard_inputs(inp, consts):
    x = np.asarray(inp["x"], np.float32)
    in_maps = []
    for j in range(NCORES):
        b, half = j // 2, j % 2
        r0 = 64 * half - 3
        slab = np.empty((C, SLAB, W), np.float32)
        slab[:] = consts["xstar"][:, None, None]
        lo, hi = max(r0, 0), min(r0 + SLAB, H)
        slab[:, lo - r0:hi - r0, :] = x[b, :, lo:hi, :]
        in_maps.append({
            "x_sh": slab,
            "blobd": consts["blobs"][half],
            "bbfd": consts["bbf"],
        })
    return in_maps


def kernel(**inputs) -> np.ndarray:
    consts = _prep_consts(inputs)
    key = "nc"
    if key not in _CACHE:
        _CACHE[key] = _build_kernel(consts)
    nc = _CACHE[key]
    in_maps = _shard_inputs(inputs, consts)
    res = run_bass_kernel_spmd(nc, in_maps, list(range(NCORES)))
    out = np.empty((B, C, H, W), np.float32)
    for j in range(NCORES):
        b, half = j // 2, j % 2
        shard = np.asarray(res.results[j]["out_sh"])
        out[b, :, 64 * half:64 * half + 64, :] = shard
    return out
